# revision 1
# baseline (speedup 1.0000x reference)
"""GAT (graph attention) message-passing kernel for Trainium2, 8 NeuronCores.

Strategy: edges sharded by destination node across cores. Host relabels nodes
(degree-balanced dealing into 128-node dst blocks) so every block has ~equal
edge count. Device phase 1 computes per-node projections h and src-scores into
a gather table (replicated per core). Phase 2 processes dst blocks: gathers
h-rows of edge sources (chunked int16 indexed dma_gather), computes softmax
weights, and aggregates via onehot-matmul into PSUM, then projects with W_out.
"""
import sys

sys.path.insert(0, "/opt/trn_rl_repo")

import numpy as np

from concourse import bacc, bass, mybir, tile
from concourse.bass_utils import run_bass_kernel_spmd

f32 = mybir.dt.float32
i16 = mybir.dt.int16
i32 = mybir.dt.int32
AF = mybir.ActivationFunctionType
ALU = mybir.AluOpType

N = 100000
E = 1600000
D = 128            # in dim
H = 4              # heads
HD = 32            # head dim
OUTD = 128
NEG = 0.2
EPS = 1e-8

NCORES = 8
BLK_PER_CORE = 98
NB_G = NCORES * BLK_PER_CORE      # 784 global blocks
NPAD = NB_G * 128                 # 100352 padded nodes
NPB = BLK_PER_CORE * 128          # 12544 nodes per core
NCHUNK = 4
CH = NPAD // NCHUNK               # 25088 rows per gather chunk (< 32768)
ROW = 192                         # table row: [h(128) | s_src(4) | pad] f32 = 768B


# ---------------------------------------------------------------- host prep
def _host_prep(x, edge_index, mask, W, a_src, a_dst, W_out):
    src = np.asarray(edge_index[0], np.int64)
    dst = np.asarray(edge_index[1], np.int64)
    m = np.asarray(mask, bool)
    keep = m[src]
    src, dst = src[keep], dst[keep]

    # node relabeling: deal nodes (sorted by in-degree desc) snake-wise into
    # NB_G blocks so block edge counts are balanced
    deg = np.bincount(dst, minlength=N)
    order = np.argsort(-deg, kind="stable")
    r = np.arange(N)
    rounds = r // NB_G
    pos = r % NB_G
    blk_of_rank = np.where(rounds % 2 == 0, pos, NB_G - 1 - pos)
    pi = np.empty(N, np.int64)
    pi[order] = blk_of_rank * 128 + rounds

    nsrc = pi[src]
    ndst = pi[dst]
    core = ndst // NPB
    b_loc = (ndst % NPB) // 128
    seg = ndst % 128
    ch = nsrc // CH
    loc = nsrc % CH

    # per (core, block, chunk) edge counts
    gid = (core * BLK_PER_CORE + b_loc) * NCHUNK + ch
    counts = np.bincount(gid, minlength=NB_G * NCHUNK).reshape(
        NCORES, BLK_PER_CORE, NCHUNK
    )
    caps = counts.max(axis=0)  # [BLK_PER_CORE, NCHUNK]
    caps = np.maximum(((caps + 127) // 128) * 128, 128).astype(np.int64)

    blk_slots = caps.sum(axis=1)              # [BLK_PER_CORE]
    blk_off = np.concatenate([[0], np.cumsum(blk_slots)])
    tot = int(blk_off[-1])
    grp_off = np.zeros((BLK_PER_CORE, NCHUNK), np.int64)
    for b in range(BLK_PER_CORE):
        o = blk_off[b]
        for c in range(NCHUNK):
            grp_off[b, c] = o
            o += caps[b, c]

    # slot assignment per core
    idx_flat = np.zeros((NCORES, tot), np.int16)      # pad -> row 0
    seg_flat = np.full((NCORES, tot), 128.0, np.float32)  # pad -> seg 128
    ordr = np.lexsort((loc, ch, b_loc, core))
    core_s, b_s, ch_s, loc_s, seg_s = (
        core[ordr], b_loc[ordr], ch[ordr], loc[ordr], seg[ordr]
    )
    # position of each edge within its (core, block, chunk) group
    gkey = (core_s * BLK_PER_CORE + b_s) * NCHUNK + ch_s
    # edges are sorted by gkey; rank within group:
    first = np.concatenate([[True], gkey[1:] != gkey[:-1]])
    gstart = np.flatnonzero(first)
    grp_len = np.diff(np.concatenate([gstart, [len(gkey)]]))
    rank = np.arange(len(gkey)) - np.repeat(gstart, grp_len)
    slot = grp_off[b_s, ch_s] + rank
    idx_flat[core_s, slot] = loc_s.astype(np.int16)
    seg_flat[core_s, slot] = seg_s.astype(np.float32)

    # device layouts
    # idxs: per (block, chunk) wrap cap idxs -> [16, cap/16] -> tile to [128, cap/16]
    idx_dev = np.zeros((NCORES, 128, tot // 16), np.int16)
    # segs: slot i -> [i%128, i//128] within block
    seg_dev = np.zeros((NCORES, 128, tot // 128), np.float32)
    # segfm: seg value per slot, replicated across 128 partitions (bf16)
    import jax.numpy as jnp
    segfm_dev = np.asarray(jnp.asarray(seg_flat, jnp.bfloat16))  # [NCORES, tot]
    segfm_rep = [np.ascontiguousarray(np.broadcast_to(segfm_dev[c][None, :], (128, tot)))
                 for c in range(NCORES)]
    for b in range(BLK_PER_CORE):
        for c in range(NCHUNK):
            o = grp_off[b, c]
            cap = caps[b, c]
            chunk_idx = idx_flat[:, o : o + cap]                 # [8, cap]
            wrap = chunk_idx.reshape(NCORES, cap // 16, 16).transpose(0, 2, 1)
            idx_dev[:, :, o // 16 : (o + cap) // 16] = np.tile(wrap, (1, 8, 1))
        o = blk_off[b]
        sl = seg_flat[:, o : o + blk_slots[b]]
        seg_dev[:, :, o // 128 : (o + blk_slots[b]) // 128] = sl.reshape(
            NCORES, blk_slots[b] // 128, 128
        ).transpose(0, 2, 1)

    # xT padded and permuted: column pi[n] holds x[n]  (bf16 for device matmuls)
    xT32 = np.zeros((D, NPAD), np.float32)
    xT32[:, pi] = np.asarray(x, np.float32).T
    import jax.numpy as jnp
    xT = np.asarray(jnp.asarray(xT32, jnp.bfloat16))

    # weights
    W_cat = np.asarray(W, np.float32).transpose(1, 0, 2).reshape(D, H * HD)
    A_src = np.zeros((H * HD, H), np.float32)
    A_dst = np.zeros((H * HD, H), np.float32)
    for h in range(H):
        A_src[h * HD : (h + 1) * HD, h] = np.asarray(a_src, np.float32)[h]
        A_dst[h * HD : (h + 1) * HD, h] = np.asarray(a_dst, np.float32)[h]
    M_src = W_cat @ A_src            # [128, 4]
    M_dst = W_cat @ A_dst            # [128, 4]
    wcat_ext = np.concatenate([W_cat, M_src], axis=1)  # [128, 132]

    meta = dict(
        caps=caps, blk_off=blk_off, grp_off=grp_off, blk_slots=blk_slots,
        tot=tot, pi=pi,
    )
    wcat_b = np.asarray(jnp.asarray(wcat_ext, jnp.bfloat16))
    mdst_b = np.asarray(jnp.asarray(M_dst, jnp.bfloat16))
    wout_b = np.asarray(jnp.asarray(np.asarray(W_out, np.float32), jnp.bfloat16))
    ident_b = np.asarray(jnp.asarray(np.eye(128, dtype=np.float32), jnp.bfloat16))
    per_core = []
    for c in range(NCORES):
        per_core.append(
            dict(
                xT=xT,
                xTd=np.ascontiguousarray(xT[:, c * NPB : (c + 1) * NPB]),
                wcat_ext=wcat_b,
                mdst=mdst_b,
                wout=wout_b,
                ident=ident_b,
                idxs=idx_dev[c],
                segs=seg_dev[c],
                segfm=segfm_rep[c],
            )
        )
    return per_core, meta


# ---------------------------------------------------------------- device build
def _build_nc(meta):
    caps = meta["caps"]
    blk_off = meta["blk_off"]
    grp_off = meta["grp_off"]
    blk_slots = meta["blk_slots"]
    tot = meta["tot"]
    bf16 = mybir.dt.bfloat16

    nc = bacc.Bacc(None, target_bir_lowering=False)
    xT = nc.dram_tensor("xT", [D, NPAD], bf16, kind="ExternalInput")
    xTd = nc.dram_tensor("xTd", [D, NPB], bf16, kind="ExternalInput")
    wcat_ext = nc.dram_tensor("wcat_ext", [D, 132], bf16, kind="ExternalInput")
    mdst = nc.dram_tensor("mdst", [D, H], bf16, kind="ExternalInput")
    wout = nc.dram_tensor("wout", [H * HD, OUTD], bf16, kind="ExternalInput")
    ident = nc.dram_tensor("ident", [128, 128], bf16, kind="ExternalInput")
    idxs = nc.dram_tensor("idxs", [128, tot // 16], i16, kind="ExternalInput")
    segs = nc.dram_tensor("segs", [128, tot // 128], f32, kind="ExternalInput")
    segfm = nc.dram_tensor("segfm", [128, tot], bf16, kind="ExternalInput")
    out = nc.dram_tensor("out", [NPB, OUTD], f32, kind="ExternalOutput")
    table = nc.dram_tensor("table", [NPAD, 256], bf16, kind="Internal")

    n_t1 = NPAD // 128  # phase-1 tiles

    with tile.TileContext(nc) as tc:
        with (
            tc.tile_pool(name="const", bufs=1) as cpool,
            tc.tile_pool(name="p1", bufs=4) as p1,
            tc.tile_pool(name="gath", bufs=2) as gp,
            tc.tile_pool(name="work", bufs=3) as wp,
            tc.tile_pool(name="outp", bufs=3) as op_,
            tc.tile_pool(name="ps1", bufs=2, space="PSUM") as ps1,
            tc.tile_pool(name="psB", bufs=2, space="PSUM") as psB,
            tc.tile_pool(name="psS", bufs=1, space="PSUM") as psS,
            tc.tile_pool(name="psT", bufs=1, space="PSUM") as psT,
            tc.tile_pool(name="psE", bufs=1, space="PSUM") as psE,
        ):
            # constants
            wcat_sb = cpool.tile([D, 132], bf16)
            nc.sync.dma_start(wcat_sb[:, :], wcat_ext[:, :])
            mdst_sb = cpool.tile([D, H], bf16)
            nc.sync.dma_start(mdst_sb[:, :], mdst[:, :])
            wout_sb = cpool.tile([H * HD, OUTD], bf16)
            nc.sync.dma_start(wout_sb[:, :], wout[:, :])
            ident_sb = cpool.tile([128, 128], bf16)
            nc.sync.dma_start(ident_sb[:, :], ident[:, :])
            iota_i = cpool.tile([128, 128], i32)
            nc.gpsimd.iota(iota_i[:, :], pattern=[[1, 128]], base=0,
                           channel_multiplier=0)
            iota_b = cpool.tile([128, 128], bf16)
            nc.vector.tensor_copy(iota_b[:, :], iota_i[:, :])
            iotac_i = cpool.tile([128, 1], i32)
            nc.gpsimd.iota(iotac_i[:, :], pattern=[[0, 1]], base=0,
                           channel_multiplier=1)
            iotac_f = cpool.tile([128, 1], f32)
            nc.vector.tensor_copy(iotac_f[:, :], iotac_i[:, :])

            # ---------------- phase 1: table[n] = [h(128) | s_src(4)] ----------
            for i in range(n_t1):
                xt_t = p1.tile([128, 128], bf16, tag="xt")
                nc.sync.dma_start(xt_t[:, :], xT[:, i * 128 : (i + 1) * 128])
                ps = ps1.tile([128, 132], f32, tag="ps1")
                nc.tensor.matmul(ps[:, :], xt_t[:, :], wcat_sb[:, :],
                                 start=True, stop=True)
                row = p1.tile([128, 132], bf16, tag="row")
                nc.vector.tensor_copy(row[:, :], ps[:, :])
                nc.sync.dma_start(table[i * 128 : (i + 1) * 128, 0:132], row[:, :])

            # ---------------- phase 2: per dst block --------------------------
            for b in range(BLK_PER_CORE):
                nt = int(blk_slots[b]) // 128  # tiles in this block
                o16 = int(blk_off[b]) // 16
                o128 = int(blk_off[b]) // 128

                it = wp.tile([128, blk_slots[b] // 16], i16, tag="it")
                nc.sync.dma_start(it[:, :], idxs[:, o16 : o16 + blk_slots[b] // 16])
                sg = wp.tile([128, nt], f32, tag="sg")
                nc.sync.dma_start(sg[:, :], segs[:, o128 : o128 + nt])
                sfm = wp.tile([128, blk_slots[b]], bf16, tag="sfm")
                nc.sync.dma_start(
                    sfm[:, :],
                    segfm[:, blk_off[b] : blk_off[b] + blk_slots[b]],
                )

                # s_dst for this block of 128 dst nodes
                xtd_t = wp.tile([128, 128], bf16, tag="xtd")
                nc.sync.dma_start(xtd_t[:, :], xTd[:, b * 128 : (b + 1) * 128])
                ps_sd = psS.tile([128, H], f32, tag="sd")
                nc.tensor.matmul(ps_sd[:, :], xtd_t[:, :], mdst_sb[:, :],
                                 start=True, stop=True)
                sdst_b = wp.tile([128, H], bf16, tag="sdst")
                nc.vector.tensor_copy(sdst_b[:, :], ps_sd[:, :])

                # gather table rows for all slots (4 chunks)
                G = gp.tile([128, nt, 256], bf16, tag="G")
                for c in range(NCHUNK):
                    cap = int(caps[b, c])
                    go = (int(grp_off[b, c]) - int(blk_off[b])) // 128
                    nc.gpsimd.dma_gather(
                        out_ap=G[:, go : go + cap // 128, :],
                        in_ap=table[c * CH : (c + 1) * CH, :],
                        idxs_ap=it[:, (int(grp_off[b, c]) - int(blk_off[b])) // 16 :
                                   (int(grp_off[b, c]) - int(blk_off[b]) + cap) // 16],
                        num_idxs=cap,
                        num_idxs_reg=cap,
                        elem_size=256,
                    )

                # ohT[j, slot] = (seg(slot) == j), one batched op per block
                ohT = wp.tile([128, blk_slots[b]], bf16, tag="ohT")
                nc.vector.tensor_scalar(ohT[:, :], sfm[:, :], iotac_f[:, 0:1],
                                        None, op0=ALU.is_equal)
                # s_dst expanded to slots via PE
                ps_se = psE.tile([128, nt * H], f32, tag="se")
                for t in range(nt):
                    nc.tensor.matmul(ps_se[:, t * H : (t + 1) * H],
                                     ohT[:, t * 128 : (t + 1) * 128],
                                     sdst_b[:, :], start=True, stop=True)

                # scores: w = exp(max(e, NEG*e)), e = s_src + s_dst  [128, nt, H]
                ssrc = wp.tile([128, nt, H], f32, tag="ssrc")
                nc.vector.tensor_copy(ssrc[:, :, :], G[:, :, 128 : 128 + H])
                esum = wp.tile([128, nt, H], f32, tag="esum")
                nc.vector.tensor_tensor(
                    esum[:, :, :], ssrc[:, :, :],
                    ps_se[:, :].rearrange("p (t h) -> p t h", h=H),
                    op=ALU.add,
                )
                e2 = wp.tile([128, nt, H], f32, tag="e2")
                nc.vector.tensor_scalar_mul(e2[:, :, :], esum[:, :, :], NEG)
                lr = wp.tile([128, nt, H], f32, tag="lr")
                nc.vector.tensor_tensor(lr[:, :, :], esum[:, :, :], e2[:, :, :],
                                        op=ALU.max)
                w = wp.tile([128, nt, H], f32, tag="w")
                nc.scalar.activation(w[:, :, :], lr[:, :, :], AF.Exp)
                wb = wp.tile([128, nt, H], bf16, tag="wb")
                nc.vector.tensor_copy(wb[:, :, :], w[:, :, :])

                # G2 = [G * w_bcast | w]  -> [128, nt, 132] bf16
                G2 = gp.tile([128, nt, 132], bf16, tag="G2")
                w_b = wb[:, :, :].unsqueeze(3).broadcast_to((128, nt, H, HD))
                nc.vector.tensor_tensor(
                    G2[:, :, 0:128].rearrange("p t (h k) -> p t h k", h=H),
                    G[:, :, 0:128].rearrange("p t (h k) -> p t h k", h=H),
                    w_b,
                    op=ALU.mult,
                )
                nc.vector.tensor_copy(G2[:, :, 128:132], wb[:, :, :])

                # aggregation: psum[seg, 0:128] = sum alpha*h ; [:,128:132] = Z
                pb = psB.tile([128, 132], f32, tag="pb")
                for t in range(nt):
                    oh = wp.tile([128, 128], bf16, tag="oh")
                    nc.vector.tensor_scalar(
                        oh[:, :], iota_b[:, :], sg[:, t : t + 1], None,
                        op0=ALU.is_equal,
                    )
                    nc.tensor.matmul(pb[:, :], oh[:, :], G2[:, t, :],
                                     start=(t == 0), stop=(t == nt - 1))

                # normalize: na = agg / (Z + eps)
                radd = wp.tile([128, H], f32, tag="radd")
                nc.vector.tensor_scalar_add(radd[:, :], pb[:, 128:132], EPS)
                rec = wp.tile([128, H], f32, tag="rec")
                nc.vector.reciprocal(rec[:, :], radd[:, :])
                na = op_.tile([128, 128], bf16, tag="na")
                nc.vector.tensor_tensor(
                    na[:, :].rearrange("p (h k) -> p h k", h=H),
                    pb[:, 0:128].rearrange("p (h k) -> p h k", h=H),
                    rec[:, :].unsqueeze(2).broadcast_to((128, H, HD)),
                    op=ALU.mult,
                )

                # out rows = (na @ wout): transpose na, then matmul
                pt = psT.tile([128, 128], bf16, tag="pt")
                nc.tensor.transpose(pt[:, :], na[:, :], ident_sb[:, :])
                naT = op_.tile([128, 128], bf16, tag="naT")
                nc.vector.tensor_copy(naT[:, :], pt[:, :])
                po = psT.tile([128, 128], f32, tag="po")
                nc.tensor.matmul(po[:, :], naT[:, :], wout_sb[:, :],
                                 start=True, stop=True)
                ot = op_.tile([128, 128], f32, tag="ot")
                nc.vector.tensor_copy(ot[:, :], po[:, :])
                nc.sync.dma_start(out[b * 128 : (b + 1) * 128, :], ot[:, :])

    nc.compile()
    return nc


# ---------------------------------------------------------------- entry point
def kernel(x, edge_index, mask, W, a_src, a_dst, W_out, _cache={}):
    per_core, meta = _host_prep(x, edge_index, mask, W, a_src, a_dst, W_out)
    key = (meta["tot"], tuple(meta["blk_slots"].tolist()))
    if key not in _cache:
        _cache[key] = _build_nc(meta)
    nc = _cache[key]
    res = run_bass_kernel_spmd(nc, per_core, core_ids=list(range(NCORES)))
    out_new = np.concatenate([res.results[c]["out"] for c in range(NCORES)], axis=0)
    return out_new[meta["pi"]].astype(np.float32)


if __name__ == "__main__":
    rng = np.random.default_rng(0)
    x = rng.standard_normal((N, D)).astype(np.float32)
    ei = rng.integers(0, N, size=(2, E)).astype(np.int32)
    mask = np.ones((N,), bool)
    W = (rng.standard_normal((H, D, HD)) * 0.05).astype(np.float32)
    a_s = (rng.standard_normal((H, HD)) * 0.1).astype(np.float32)
    a_d = (rng.standard_normal((H, HD)) * 0.1).astype(np.float32)
    W_o = (rng.standard_normal((H * HD, OUTD)) * 0.05).astype(np.float32)
    out = kernel(x, ei, mask, W, a_s, a_d, W_o)
    print("ok", out.shape, out.dtype)



# revision 9
# speedup vs baseline: 4.1536x; 4.1536x over previous
"""GAT (graph attention) message-passing kernel for Trainium2, 8 NeuronCores.

v3: gather-free edge-expanded streaming. Host pre-expands x into edge-slot
order (pure indexing): nodes are relabeled by in-degree (desc) so each
128-node dst block has near-uniform degree; slots are seg-aligned (partition
p of a tile holds only edges of dst p in the block), so segment softmax and
aggregation become per-partition ops with no one-hots and no device gather.
Device streams x-slot tiles: one matmul per tile produces [h | s_src] in
PSUM, an identity-stationary matmul adds s_dst, leaky-relu + exp on
vector/scalar engines produce edge weights, gpsimd multiplies messages, a
vector reduce aggregates per dst, and a transpose + matmul applies W_out.
Pad slots use a host-computed vector v with v@M_src = -1e3 so their weight
underflows to exactly zero.
"""
import sys

sys.path.insert(0, "/opt/trn_rl_repo")

import numpy as np

from concourse import bacc, bass, mybir, tile
from concourse.bass_utils import run_bass_kernel_spmd

f32 = mybir.dt.float32
bf16 = mybir.dt.bfloat16
AF = mybir.ActivationFunctionType
ALU = mybir.AluOpType

N = 100000
E = 1600000
D = 128            # in dim
H = 4              # heads
HD = 32            # head dim
OUTD = 128
NEG = 0.2
EPS = 1e-8

NCORES = 8
BLK_PER_CORE = 98
NB_G = NCORES * BLK_PER_CORE      # 784 global blocks
NPAD = NB_G * 128                 # 100352 padded nodes
NPB = BLK_PER_CORE * 128          # 12544 dst nodes per core
WIN = 3                           # tiles per PSUM window (132*3*4B < 2KB bank)
SK = 3                            # windows per PSUM super-tile (banks)


# ---------------------------------------------------------------- host prep
def _host_prep(x, edge_index, mask, W, a_src, a_dst, W_out):
    import jax.numpy as jnp

    src = np.asarray(edge_index[0], np.int64)
    dst = np.asarray(edge_index[1], np.int64)
    m = np.asarray(mask, bool)
    keep = m[src]
    src, dst = src[keep], dst[keep]

    # nodes sorted by in-degree desc; block k = sorted[128k:128k+128]
    deg = np.bincount(dst, minlength=N)
    order = np.argsort(-deg, kind="stable")      # newid -> node
    newid = np.empty(N, np.int64)
    newid[order] = np.arange(N)                  # node -> newid

    deg_sorted = deg[order]
    maxdeg_blk = np.zeros(NB_G, np.int64)
    maxdeg_blk[: (N + 127) // 128] = deg_sorted[
        np.minimum(np.arange((N + 127) // 128) * 128, N - 1)
    ]

    # snake deal global blocks to cores: round r covers blocks 8r..8r+7
    # core c's k-th block: b = 8k + (c if k even else 7-c)
    ks = np.arange(BLK_PER_CORE)
    b_of = np.empty((NCORES, BLK_PER_CORE), np.int64)
    for c in range(NCORES):
        b_of[c] = 8 * ks + np.where(ks % 2 == 0, c, 7 - c)
    core_of_blk = np.empty(NB_G, np.int64)
    k_of_blk = np.empty(NB_G, np.int64)
    for c in range(NCORES):
        core_of_blk[b_of[c]] = c
        k_of_blk[b_of[c]] = ks

    # per-k nt shared across cores (single compiled kernel)
    nt_k = np.zeros(BLK_PER_CORE, np.int64)
    for k in range(BLK_PER_CORE):
        nt_k[k] = maxdeg_blk[b_of[:, k]].max()
    nt_k = np.maximum(((nt_k + WIN - 1) // WIN) * WIN, WIN)
    blk_off = np.concatenate([[0], np.cumsum(nt_k * 128)])
    TOT = int(blk_off[-1])

    # per-edge slot position: sort by new dst id, rank within dst
    ndst = newid[dst]
    ordr = np.argsort(ndst, kind="stable")
    ndst_s, src_s = ndst[ordr], src[ordr]
    first = np.concatenate([[True], ndst_s[1:] != ndst_s[:-1]])
    gstart = np.flatnonzero(first)
    grp_len = np.diff(np.concatenate([gstart, [len(ndst_s)]]))
    rank = np.arange(len(ndst_s)) - np.repeat(gstart, grp_len)

    blk = ndst_s // 128
    p = ndst_s % 128
    core_e = core_of_blk[blk]
    k_e = k_of_blk[blk]
    col = blk_off[k_e] + rank * 128 + p

    # column maps (N -> pad-src v-row, N+1 -> zero row)
    colmap = np.full((NCORES, TOT), N, np.int64)
    colmap[core_e, col] = src_s

    # dst node map for s_dst (zero row for virtual pad nodes)
    dstmap = np.full((NCORES, NPB), N + 1, np.int64)
    for c in range(NCORES):
        gb = b_of[c]                              # 98 global block ids
        nid = (gb[:, None] * 128 + np.arange(128)[None, :]).reshape(-1)
        valid = nid < N
        dstmap[c][valid] = order[nid[valid]]

    # output row of each node
    pi = np.empty(N, np.int64)
    for c in range(NCORES):
        gb = b_of[c]
        nid = (gb[:, None] * 128 + np.arange(128)[None, :]).reshape(-1)
        valid = nid < N
        rows = c * NPB + np.arange(NPB)
        pi[order[nid[valid]]] = rows[valid]

    # weights
    Wf = np.asarray(W, np.float32)
    Wcat = Wf.transpose(1, 0, 2).reshape(D, H * HD)        # [128,128]
    asrc = np.asarray(a_src, np.float32)
    adst = np.asarray(a_dst, np.float32)
    Msrc = np.stack([Wcat[:, h * HD:(h + 1) * HD] @ asrc[h] for h in range(H)], 1)
    Mdst = np.stack([Wcat[:, h * HD:(h + 1) * HD] @ adst[h] for h in range(H)], 1)
    # pad vector: v @ Msrc = -1000 for every head
    v = np.linalg.lstsq(Msrc.T, np.full(H, -1000.0, np.float32), rcond=None)[0]

    wcat_ext = np.concatenate([Wcat, Msrc], 1)             # [128,132]

    def tobf(a):
        return np.asarray(jnp.asarray(np.asarray(a, np.float32), jnp.bfloat16))

    # x extended: rows 0..N-1 = x, N = v (src pad), N+1 = 0 (dst pad)
    x_ext = np.zeros((N + 2, D), np.float32)
    x_ext[:N] = np.asarray(x, np.float32)
    x_ext[N] = v
    xT_ext = tobf(x_ext).T                                  # bf16 [128, N+2]
    xT_u16 = np.ascontiguousarray(xT_ext).view(np.uint16)

    wcat_b = tobf(wcat_ext)
    mdst_b = tobf(Mdst)
    wout_b = tobf(np.asarray(W_out, np.float32))
    ident_b = tobf(np.eye(128, dtype=np.float32))

    per_core = []
    for c in range(NCORES):
        xs = np.take(xT_u16, colmap[c], axis=1)             # [128, TOT] u16
        xd = np.take(xT_u16, dstmap[c], axis=1)             # [128, NPB] u16
        per_core.append(
            dict(
                xslots=xs.view(xT_ext.dtype),
                xtd=xd.view(xT_ext.dtype),
                wcat_ext=wcat_b,
                mdst=mdst_b,
                wout=wout_b,
                ident=ident_b,
            )
        )
    meta = dict(nt_k=nt_k, blk_off=blk_off, tot=TOT, pi=pi)
    return per_core, meta


# ---------------------------------------------------------------- device build
def _build_nc(meta):
    nt_k = meta["nt_k"]
    blk_off = meta["blk_off"]
    TOT = meta["tot"]

    nc = bacc.Bacc(None, target_bir_lowering=False)
    xslots = nc.dram_tensor("xslots", [D, TOT], bf16, kind="ExternalInput")
    xtd = nc.dram_tensor("xtd", [D, NPB], bf16, kind="ExternalInput")
    wcat_ext = nc.dram_tensor("wcat_ext", [D, 132], bf16, kind="ExternalInput")
    mdst = nc.dram_tensor("mdst", [D, H], bf16, kind="ExternalInput")
    wout = nc.dram_tensor("wout", [H * HD, OUTD], bf16, kind="ExternalInput")
    ident = nc.dram_tensor("ident", [128, 128], bf16, kind="ExternalInput")
    out = nc.dram_tensor("out", [NPB, OUTD], f32, kind="ExternalOutput")

    with tile.TileContext(nc) as tc:
        with (
            tc.tile_pool(name="const", bufs=1) as cpool,
            tc.tile_pool(name="xin", bufs=3) as xp,
            tc.tile_pool(name="g2", bufs=2) as g2p,
            tc.tile_pool(name="wk", bufs=3) as wp,
            tc.tile_pool(name="outp", bufs=3) as op_,
            tc.tile_pool(name="psW", bufs=2, space="PSUM") as psW_,
            tc.tile_pool(name="psO", bufs=1, space="PSUM") as psO_,
        ):
            wcat_sb = cpool.tile([D, 132], bf16)
            nc.sync.dma_start(wcat_sb[:, :], wcat_ext[:, :])
            mdst_sb = cpool.tile([D, H], bf16)
            nc.sync.dma_start(mdst_sb[:, :], mdst[:, :])
            wout_sb = cpool.tile([H * HD, OUTD], bf16)
            nc.sync.dma_start(wout_sb[:, :], wout[:, :])
            ident_sb = cpool.tile([128, 128], bf16)
            nc.sync.dma_start(ident_sb[:, :], ident[:, :])

            alneg = cpool.tile([128, 1], f32)
            nc.vector.memset(alneg[:, :], NEG)

            # s_dst for all blocks: [128, 98, 4] bf16
            sdst_all = cpool.tile([128, BLK_PER_CORE, H], bf16)
            xtd_sb = cpool.tile([128, NPB // 128, 128], bf16)
            nc.sync.dma_start(
                xtd_sb[:, :, :],
                xtd[:, :].rearrange("d (k p) -> d k p", p=128),
            )
            for k4 in range(0, BLK_PER_CORE, 4):
                kk = min(4, BLK_PER_CORE - k4)
                ps_sd = psO_.tile([128, 4, H], f32, tag="po")
                for j in range(kk):
                    nc.tensor.matmul(ps_sd[:, j, :], xtd_sb[:, k4 + j, :],
                                     mdst_sb[:, :], start=True, stop=True)
                nc.scalar.copy(sdst_all[:, k4 : k4 + kk, :], ps_sd[:, 0:kk, :])

            # main block loop
            for k in range(BLK_PER_CORE):
                nt = int(nt_k[k])
                nwin = nt // WIN
                off = int(blk_off[k])

                xslab = xp.tile([128, nt, 128], bf16, tag="xslab")
                nc.sync.dma_start(
                    xslab[:, :, :],
                    xslots[:, off : off + nt * 128].rearrange(
                        "d (t p) -> d t p", p=128
                    ),
                )

                G2 = g2p.tile([128, nwin, 132, WIN], bf16, tag="G2")
                sd12 = wp.tile([128, H, WIN], bf16, tag="sd12")
                nc.vector.tensor_copy(
                    sd12[:, :, :],
                    sdst_all[:, k, :].unsqueeze(2).broadcast_to((128, H, WIN)),
                )
                # super-windows of up to SK windows (SK psum banks)
                for s in range(0, nwin, SK):
                    kk = min(SK, nwin - s)
                    psW = psW_.tile([128, kk, 512], f32, tag="psW")
                    for w2 in range(kk):
                        pw = psW[:, w2, 0 : 132 * WIN].rearrange(
                            "p (c t) -> p c t", t=WIN)
                        for j in range(WIN):
                            nc.tensor.matmul(pw[:, :, j],
                                             xslab[:, (s + w2) * WIN + j, :],
                                             wcat_sb[:, :],
                                             start=True, stop=True)
                    e12 = wp.tile([128, kk, H, WIN], f32, tag="e12")
                    nc.vector.scalar_tensor_tensor(
                        e12[:, :, :, :],
                        psW[:, :, 128 * WIN : 132 * WIN].rearrange(
                            "p k (c t) -> p k c t", t=WIN),
                        1.0,
                        sd12[:, :, :].unsqueeze(1).broadcast_to(
                            (128, kk, H, WIN)),
                        op0=ALU.mult, op1=ALU.add,
                    )
                    lr = wp.tile([128, kk, H, WIN], f32, tag="lr")
                    nc.scalar.activation(lr[:, :, :, :], e12[:, :, :, :],
                                         AF.Prelu, alpha=alneg[:, 0:1])
                    wexp = wp.tile([128, kk, 128, WIN], bf16, tag="wexp")
                    nc.scalar.activation(
                        wexp[:, :, :, :].rearrange(
                            "p k (h x) t -> p k h x t", h=H),
                        lr[:, :, :, :].unsqueeze(3).broadcast_to(
                            (128, kk, H, HD, WIN)),
                        AF.Exp,
                    )
                    nc.scalar.activation(G2[:, s : s + kk, 128:132, :],
                                         lr[:, :, :, :], AF.Exp)
                    nc.vector.tensor_tensor(
                        G2[:, s : s + kk, 0:128, :],
                        psW[:, :, 0 : 128 * WIN].rearrange(
                            "p k (c t) -> p k c t", t=WIN),
                        wexp[:, :, :, :], op=ALU.mult,
                    )

                # aggregate: pb[p, c] = sum over (win, t)
                pb = wp.tile([128, 132], f32, tag="pb")
                nc.vector.tensor_reduce(
                    pb[:, :],
                    G2[:, :, :, :].rearrange("p w c t -> p c w t"),
                    mybir.AxisListType.XY, ALU.add,
                )
                radd = wp.tile([128, H], f32, tag="radd")
                nc.vector.tensor_scalar_add(radd[:, :], pb[:, 128:132], EPS)
                rec = wp.tile([128, H], f32, tag="rec")
                nc.vector.reciprocal(rec[:, :], radd[:, :])
                na = op_.tile([128, 128], bf16, tag="na")
                nc.vector.tensor_tensor(
                    na[:, :].rearrange("p (h x) -> p h x", h=H),
                    pb[:, 0:128].rearrange("p (h x) -> p h x", h=H),
                    rec[:, :].unsqueeze(2).broadcast_to((128, H, HD)),
                    op=ALU.mult,
                )
                pt = psO_.tile([128, 128], bf16, tag="pt")
                nc.tensor.transpose(pt[:, :], na[:, :], ident_sb[:, :])
                naT = op_.tile([128, 128], bf16, tag="naT")
                nc.scalar.copy(naT[:, :], pt[:, :])
                po = psO_.tile([128, 128], f32, tag="po")
                nc.tensor.matmul(po[:, :], naT[:, :], wout_sb[:, :],
                                 start=True, stop=True)
                ot = op_.tile([128, 128], f32, tag="ot")
                nc.vector.tensor_copy(ot[:, :], po[:, :])
                nc.sync.dma_start(out[k * 128 : (k + 1) * 128, :], ot[:, :])

    nc.compile()
    return nc


# ---------------------------------------------------------------- entry point
def kernel(x, edge_index, mask, W, a_src, a_dst, W_out, _cache={}):
    per_core, meta = _host_prep(x, edge_index, mask, W, a_src, a_dst, W_out)
    key = (meta["tot"], tuple(meta["nt_k"].tolist()))
    if key not in _cache:
        _cache[key] = _build_nc(meta)
    nc = _cache[key]
    res = run_bass_kernel_spmd(nc, per_core, core_ids=list(range(NCORES)))
    out_new = np.concatenate([res.results[c]["out"] for c in range(NCORES)], axis=0)
    return out_new[meta["pi"]].astype(np.float32)


if __name__ == "__main__":
    rng = np.random.default_rng(0)
    x = rng.standard_normal((N, D)).astype(np.float32)
    ei = rng.integers(0, N, size=(2, E)).astype(np.int32)
    mask = np.ones((N,), bool)
    Wt = (rng.standard_normal((H, D, HD)) * 0.05).astype(np.float32)
    a_s = (rng.standard_normal((H, HD)) * 0.1).astype(np.float32)
    a_d = (rng.standard_normal((H, HD)) * 0.1).astype(np.float32)
    W_o = (rng.standard_normal((H * HD, OUTD)) * 0.05).astype(np.float32)
    out = kernel(x, ei, mask, Wt, a_s, a_d, W_o)
    print("ok", out.shape, out.dtype)


# revision 11
# speedup vs baseline: 4.3799x; 1.0545x over previous
"""GAT (graph attention) message-passing kernel for Trainium2, 8 NeuronCores.

v3: gather-free edge-expanded streaming. Host pre-expands x into edge-slot
order (pure indexing): nodes are relabeled by in-degree (desc) so each
128-node dst block has near-uniform degree; slots are seg-aligned (partition
p of a tile holds only edges of dst p in the block), so segment softmax and
aggregation become per-partition ops with no one-hots and no device gather.
Device streams x-slot tiles: one matmul per tile produces [h | s_src] in
PSUM, an identity-stationary matmul adds s_dst, leaky-relu + exp on
vector/scalar engines produce edge weights, gpsimd multiplies messages, a
vector reduce aggregates per dst, and a transpose + matmul applies W_out.
Pad slots use a host-computed vector v with v@M_src = -1e3 so their weight
underflows to exactly zero.
"""
import sys

sys.path.insert(0, "/opt/trn_rl_repo")

import numpy as np

from concourse import bacc, bass, mybir, tile
from concourse.bass_utils import run_bass_kernel_spmd

f32 = mybir.dt.float32
bf16 = mybir.dt.bfloat16
AF = mybir.ActivationFunctionType
ALU = mybir.AluOpType

N = 100000
E = 1600000
D = 128            # in dim
H = 4              # heads
HD = 32            # head dim
OUTD = 128
NEG = 0.2
EPS = 1e-8

NCORES = 8
BLK_PER_CORE = 98
NB_G = NCORES * BLK_PER_CORE      # 784 global blocks
NPAD = NB_G * 128                 # 100352 padded nodes
NPB = BLK_PER_CORE * 128          # 12544 dst nodes per core
WIN = 3                           # tiles per PSUM window (132*3*4B < 2KB bank)
SK = 3                            # windows per PSUM super-tile (banks)


# ---------------------------------------------------------------- host prep
def _host_prep(x, edge_index, mask, W, a_src, a_dst, W_out):
    import jax.numpy as jnp

    src = np.asarray(edge_index[0], np.int64)
    dst = np.asarray(edge_index[1], np.int64)
    m = np.asarray(mask, bool)
    keep = m[src]
    src, dst = src[keep], dst[keep]

    # nodes sorted by in-degree desc; block k = sorted[128k:128k+128]
    deg = np.bincount(dst, minlength=N)
    order = np.argsort(-deg, kind="stable")      # newid -> node
    newid = np.empty(N, np.int64)
    newid[order] = np.arange(N)                  # node -> newid

    deg_sorted = deg[order]
    maxdeg_blk = np.zeros(NB_G, np.int64)
    maxdeg_blk[: (N + 127) // 128] = deg_sorted[
        np.minimum(np.arange((N + 127) // 128) * 128, N - 1)
    ]

    # snake deal global blocks to cores: round r covers blocks 8r..8r+7
    # core c's k-th block: b = 8k + (c if k even else 7-c)
    ks = np.arange(BLK_PER_CORE)
    b_of = np.empty((NCORES, BLK_PER_CORE), np.int64)
    for c in range(NCORES):
        b_of[c] = 8 * ks + np.where(ks % 2 == 0, c, 7 - c)
    core_of_blk = np.empty(NB_G, np.int64)
    k_of_blk = np.empty(NB_G, np.int64)
    for c in range(NCORES):
        core_of_blk[b_of[c]] = c
        k_of_blk[b_of[c]] = ks

    # per-k nt shared across cores (single compiled kernel)
    nt_k = np.zeros(BLK_PER_CORE, np.int64)
    for k in range(BLK_PER_CORE):
        nt_k[k] = maxdeg_blk[b_of[:, k]].max()
    nt_k = np.maximum(((nt_k + WIN - 1) // WIN) * WIN, WIN)
    blk_off = np.concatenate([[0], np.cumsum(nt_k * 128)])
    TOT = int(blk_off[-1])

    # per-edge slot position: sort by new dst id, rank within dst
    ndst = newid[dst]
    ordr = np.argsort(ndst, kind="stable")
    ndst_s, src_s = ndst[ordr], src[ordr]
    first = np.concatenate([[True], ndst_s[1:] != ndst_s[:-1]])
    gstart = np.flatnonzero(first)
    grp_len = np.diff(np.concatenate([gstart, [len(ndst_s)]]))
    rank = np.arange(len(ndst_s)) - np.repeat(gstart, grp_len)

    blk = ndst_s // 128
    p = ndst_s % 128
    core_e = core_of_blk[blk]
    k_e = k_of_blk[blk]
    col = blk_off[k_e] + rank * 128 + p

    # column maps (N -> pad-src v-row, N+1 -> zero row)
    colmap = np.full((NCORES, TOT), N, np.int64)
    colmap[core_e, col] = src_s

    # dst node map for s_dst (zero row for virtual pad nodes)
    dstmap = np.full((NCORES, NPB), N + 1, np.int64)
    for c in range(NCORES):
        gb = b_of[c]                              # 98 global block ids
        nid = (gb[:, None] * 128 + np.arange(128)[None, :]).reshape(-1)
        valid = nid < N
        dstmap[c][valid] = order[nid[valid]]

    # output row of each node
    pi = np.empty(N, np.int64)
    for c in range(NCORES):
        gb = b_of[c]
        nid = (gb[:, None] * 128 + np.arange(128)[None, :]).reshape(-1)
        valid = nid < N
        rows = c * NPB + np.arange(NPB)
        pi[order[nid[valid]]] = rows[valid]

    # weights
    Wf = np.asarray(W, np.float32)
    Wcat = Wf.transpose(1, 0, 2).reshape(D, H * HD)        # [128,128]
    asrc = np.asarray(a_src, np.float32)
    adst = np.asarray(a_dst, np.float32)
    Msrc = np.stack([Wcat[:, h * HD:(h + 1) * HD] @ asrc[h] for h in range(H)], 1)
    Mdst = np.stack([Wcat[:, h * HD:(h + 1) * HD] @ adst[h] for h in range(H)], 1)
    # pad vector: v @ Msrc = -1000 for every head
    v = np.linalg.lstsq(Msrc.T, np.full(H, -1000.0, np.float32), rcond=None)[0]

    wcat_ext = np.concatenate([Wcat, Msrc], 1)             # [128,132]

    def tobf(a):
        return np.asarray(jnp.asarray(np.asarray(a, np.float32), jnp.bfloat16))

    # x extended: rows 0..N-1 = x, N = v (src pad), N+1 = 0 (dst pad)
    x_ext = np.zeros((N + 2, D), np.float32)
    x_ext[:N] = np.asarray(x, np.float32)
    x_ext[N] = v
    xT_ext = tobf(x_ext).T                                  # bf16 [128, N+2]
    xT_u16 = np.ascontiguousarray(xT_ext).view(np.uint16)

    wcat_b = tobf(wcat_ext)
    mdst_b = tobf(Mdst)
    wout_b = tobf(np.asarray(W_out, np.float32))
    ident_b = tobf(np.eye(128, dtype=np.float32))

    per_core = []
    for c in range(NCORES):
        xs = np.take(xT_u16, colmap[c], axis=1)             # [128, TOT] u16
        xd = np.take(xT_u16, dstmap[c], axis=1)             # [128, NPB] u16
        per_core.append(
            dict(
                xslots=xs.view(xT_ext.dtype),
                xtd=xd.view(xT_ext.dtype),
                wcat_ext=wcat_b,
                mdst=mdst_b,
                wout=wout_b,
                ident=ident_b,
            )
        )
    meta = dict(nt_k=nt_k, blk_off=blk_off, tot=TOT, pi=pi)
    return per_core, meta


# ---------------------------------------------------------------- device build
def _build_nc(meta):
    nt_k = meta["nt_k"]
    blk_off = meta["blk_off"]
    TOT = meta["tot"]

    nc = bacc.Bacc(None, target_bir_lowering=False)
    xslots = nc.dram_tensor("xslots", [D, TOT], bf16, kind="ExternalInput")
    xtd = nc.dram_tensor("xtd", [D, NPB], bf16, kind="ExternalInput")
    wcat_ext = nc.dram_tensor("wcat_ext", [D, 132], bf16, kind="ExternalInput")
    mdst = nc.dram_tensor("mdst", [D, H], bf16, kind="ExternalInput")
    wout = nc.dram_tensor("wout", [H * HD, OUTD], bf16, kind="ExternalInput")
    ident = nc.dram_tensor("ident", [128, 128], bf16, kind="ExternalInput")
    out = nc.dram_tensor("out", [NPB, OUTD], f32, kind="ExternalOutput")

    with tile.TileContext(nc) as tc:
        with (
            tc.tile_pool(name="const", bufs=1) as cpool,
            tc.tile_pool(name="xin", bufs=3) as xp,
            tc.tile_pool(name="g2", bufs=2) as g2p,
            tc.tile_pool(name="wk", bufs=3) as wp,
            tc.tile_pool(name="outp", bufs=3) as op_,
            tc.tile_pool(name="psW", bufs=2, space="PSUM") as psW_,
            tc.tile_pool(name="psO", bufs=1, space="PSUM") as psO_,
        ):
            wcat_sb = cpool.tile([D, 132], bf16)
            nc.sync.dma_start(wcat_sb[:, :], wcat_ext[:, :])
            mdst_sb = cpool.tile([D, H], bf16)
            nc.sync.dma_start(mdst_sb[:, :], mdst[:, :])
            wout_sb = cpool.tile([H * HD, OUTD], bf16)
            nc.sync.dma_start(wout_sb[:, :], wout[:, :])
            ident_sb = cpool.tile([128, 128], bf16)
            nc.sync.dma_start(ident_sb[:, :], ident[:, :])

            alneg = cpool.tile([128, 1], f32)
            nc.vector.memset(alneg[:, :], NEG)

            # s_dst for all blocks: [128, 98, 4] bf16
            sdst_all = cpool.tile([128, BLK_PER_CORE, H], bf16)
            xtd_sb = cpool.tile([128, NPB // 128, 128], bf16)
            nc.sync.dma_start(
                xtd_sb[:, :, :],
                xtd[:, :].rearrange("d (k p) -> d k p", p=128),
            )
            for k4 in range(0, BLK_PER_CORE, 4):
                kk = min(4, BLK_PER_CORE - k4)
                ps_sd = psO_.tile([128, 4, H], f32, tag="po")
                for j in range(kk):
                    nc.tensor.matmul(ps_sd[:, j, :], xtd_sb[:, k4 + j, :],
                                     mdst_sb[:, :], start=True, stop=True)
                nc.scalar.copy(sdst_all[:, k4 : k4 + kk, :], ps_sd[:, 0:kk, :])

            # main block loop
            for k in range(BLK_PER_CORE):
                nt = int(nt_k[k])
                nwin = nt // WIN
                off = int(blk_off[k])

                xslab = xp.tile([128, nt, 128], bf16, tag="xslab")
                nc.sync.dma_start(
                    xslab[:, :, :],
                    xslots[:, off : off + nt * 128].rearrange(
                        "d (t p) -> d t p", p=128
                    ),
                )

                # layouts: psW window [t, c] (contiguous matmul outs);
                # G2 block [p, w, t, 132]; lr/e12 [p, kk, t, H]
                G2 = g2p.tile([128, nwin, WIN, 132], bf16, tag="G2")
                sd12 = wp.tile([128, WIN, H], bf16, tag="sd12")
                nc.vector.tensor_copy(
                    sd12[:, :, :],
                    sdst_all[:, k, :].unsqueeze(1).broadcast_to((128, WIN, H)),
                )
                # super-windows of up to SK windows (SK psum banks)
                for s in range(0, nwin, SK):
                    kk = min(SK, nwin - s)
                    psW = psW_.tile([128, kk, 512], f32, tag="psW")
                    for w2 in range(kk):
                        pw = psW[:, w2, 0 : 132 * WIN].rearrange(
                            "p (t c) -> p t c", t=WIN)
                        for j in range(WIN):
                            nc.tensor.matmul(pw[:, j, :],
                                             xslab[:, (s + w2) * WIN + j, :],
                                             wcat_sb[:, :],
                                             start=True, stop=True)
                    e12 = wp.tile([128, kk, WIN, H], f32, tag="e12")
                    nc.vector.tensor_tensor(
                        e12[:, :, :, :],
                        psW[:, :, 0 : 132 * WIN].rearrange(
                            "p k (t c) -> p k t c", t=WIN)[:, :, :, 128:132],
                        sd12[:, :, :].unsqueeze(1).broadcast_to(
                            (128, kk, WIN, H)),
                        op=ALU.add,
                    )
                    lr = wp.tile([128, kk, WIN, H], f32, tag="lr")
                    nc.scalar.activation(lr[:, :, :, :], e12[:, :, :, :],
                                         AF.Prelu, alpha=alneg[:, 0:1])
                    wexp = wp.tile([128, kk, WIN, 128], bf16, tag="wexp")
                    nc.scalar.activation(
                        wexp[:, :, :, :].rearrange(
                            "p k t (h x) -> p k t h x", h=H),
                        lr[:, :, :, :].unsqueeze(4).broadcast_to(
                            (128, kk, WIN, H, HD)),
                        AF.Exp,
                    )
                    nc.scalar.activation(G2[:, s : s + kk, :, 128:132],
                                         lr[:, :, :, :], AF.Exp)
                    nc.vector.tensor_tensor(
                        G2[:, s : s + kk, :, 0:128],
                        psW[:, :, 0 : 132 * WIN].rearrange(
                            "p k (t c) -> p k t c", t=WIN)[:, :, :, 0:128],
                        wexp[:, :, :, :], op=ALU.mult,
                    )

                # aggregate: t-sum on gpsimd, then w-reduce on vector
                if WIN == 3:
                    t01 = wp.tile([128, nwin, 132], bf16, tag="t01")
                    nc.gpsimd.tensor_tensor(t01[:, :, :], G2[:, :, 0, :],
                                            G2[:, :, 1, :], op=ALU.add)
                    tsum = wp.tile([128, nwin, 132], f32, tag="tsum")
                    nc.gpsimd.tensor_tensor(tsum[:, :, :], t01[:, :, :],
                                            G2[:, :, 2, :], op=ALU.add)
                else:
                    raise NotImplementedError
                pb = wp.tile([128, 132], f32, tag="pb")
                nc.vector.tensor_reduce(
                    pb[:, :],
                    tsum[:, :, :].rearrange("p w c -> p c w"),
                    mybir.AxisListType.X, ALU.add,
                )
                radd = wp.tile([128, H], f32, tag="radd")
                nc.vector.tensor_scalar_add(radd[:, :], pb[:, 128:132], EPS)
                rec = wp.tile([128, H], f32, tag="rec")
                nc.vector.reciprocal(rec[:, :], radd[:, :])
                na = op_.tile([128, 128], bf16, tag="na")
                nc.gpsimd.tensor_tensor(
                    na[:, :].rearrange("p (h x) -> p h x", h=H),
                    pb[:, 0:128].rearrange("p (h x) -> p h x", h=H),
                    rec[:, :].unsqueeze(2).broadcast_to((128, H, HD)),
                    op=ALU.mult,
                )
                pt = psO_.tile([128, 128], bf16, tag="pt")
                nc.tensor.transpose(pt[:, :], na[:, :], ident_sb[:, :])
                naT = op_.tile([128, 128], bf16, tag="naT")
                nc.scalar.copy(naT[:, :], pt[:, :])
                po = psO_.tile([128, 128], f32, tag="po")
                nc.tensor.matmul(po[:, :], naT[:, :], wout_sb[:, :],
                                 start=True, stop=True)
                ot = op_.tile([128, 128], f32, tag="ot")
                nc.scalar.copy(ot[:, :], po[:, :])
                nc.sync.dma_start(out[k * 128 : (k + 1) * 128, :], ot[:, :])

    nc.compile()
    return nc


# ---------------------------------------------------------------- entry point
def kernel(x, edge_index, mask, W, a_src, a_dst, W_out, _cache={}):
    per_core, meta = _host_prep(x, edge_index, mask, W, a_src, a_dst, W_out)
    key = (meta["tot"], tuple(meta["nt_k"].tolist()))
    if key not in _cache:
        _cache[key] = _build_nc(meta)
    nc = _cache[key]
    res = run_bass_kernel_spmd(nc, per_core, core_ids=list(range(NCORES)))
    out_new = np.concatenate([res.results[c]["out"] for c in range(NCORES)], axis=0)
    return out_new[meta["pi"]].astype(np.float32)


if __name__ == "__main__":
    rng = np.random.default_rng(0)
    x = rng.standard_normal((N, D)).astype(np.float32)
    ei = rng.integers(0, N, size=(2, E)).astype(np.int32)
    mask = np.ones((N,), bool)
    Wt = (rng.standard_normal((H, D, HD)) * 0.05).astype(np.float32)
    a_s = (rng.standard_normal((H, HD)) * 0.1).astype(np.float32)
    a_d = (rng.standard_normal((H, HD)) * 0.1).astype(np.float32)
    W_o = (rng.standard_normal((H * HD, OUTD)) * 0.05).astype(np.float32)
    out = kernel(x, ei, mask, Wt, a_s, a_d, W_o)
    print("ok", out.shape, out.dtype)
